# revision 1
# baseline (speedup 1.0000x reference)
"""Trainium2 Bass kernel for nn_Decoder (worker/task label-probability decoder).

Math:
    worker_feature = inputs[:2048, :64]          # [Wn, A]
    tau            = inputs[2048:, :16]          # [T, L]
    p1 = sigmoid(worker_feature @ W + b)         # [Wn, 1]
    p2 = (1 - p1) / (L - 1)
    P[i, j, l] = p1[i]^tau[j,l] * p2[i]^(1 - tau[j,l])
               = exp(a[i] * tau[j,l] + c[i]),  a = ln p1 - ln p2, c = ln p2

Sharding: pure data parallel over the worker axis (dim 0), 256 workers per
core across 8 cores; tau/W/b replicated. No communication.

Per-core schedule: workers live on SBUF partitions (2 groups of 128), the
flattened task axis streams through PSUM in 2048-column tiles. tau arrives
as an exact 3-term bf16 split laid out [80, 2048] (hi/mid/lo stripe blocks
at partitions 0/32/64, one 2048-wide stripe per partition row) so the whole
thing loads in a single ~1.6us DMA. The tensor engine replicates each
stripe to all 128 partitions with one selector matmul per 512 columns
(sel3 picks the stripe's hi+mid+lo rows, summing the split exactly). The
Exp(a*tau + c) activations run on ACT with per-partition scale/bias - ACT
is the critical resource at ~0.9ns/col - while the output streams to HBM
in 2048-column writes round-robined over the SP and Pool DMA queues so
neither queue exceeds the ACT budget.
"""

import numpy as np

try:
    import concourse.bass as bass  # noqa: F401
except ImportError:  # fall back to the container's repo checkout
    import sys

    for _p in ("/root/.axon_site/_ro/trn_rl_repo", "/opt/trn_rl_repo"):
        if _p not in sys.path:
            sys.path.append(_p)

import concourse.bass as bass
import concourse.tile as tile
from concourse import mybir
from concourse.bass_utils import run_bass_kernel_spmd

WN = 2048  # workers total
TN = 2048  # tasks
L = 16  # edge types / labels
A = 64  # ability features
AA = A + 1  # features + bias column folded in
NCORES = 8
WPC = WN // NCORES  # workers per core (256)
G = WPC // 128  # partition groups per core (2)
F = TN * L  # flattened task axis (32768)

NST = 16  # tau stripes
STW = F // NST  # stripe width (2048)
MM = 512  # matmul columns per instruction (one PSUM bank)
PSW = 2048  # psum tile width (4 banks)
SPL = 80  # tau3/sel3 partition extent (hi@0, mid@32, lo@64)

_AF = mybir.ActivationFunctionType
_f32 = mybir.dt.float32
_bf16 = mybir.dt.bfloat16

WRITE_ENGINES = ("sync", "gpsimd")


class _TC(tile.TileContext):
    """TileContext legalized for a walrus that allows one sync-wait per inst.

    After Tile's normal scheduling + the exit drain/barrier, rewrite every
    multi-wait instruction into a chain of same-engine NOPs (one wait each)
    followed by the instruction with the final wait.
    """

    def _drain_and_barrier(self, tick_clock, wait_clock):
        super()._drain_and_barrier(tick_clock, wait_clock)
        self._split_multi_waits()

    def _fresh_nop(self, engine):
        inst = self.nc.engines[engine].nop(nofuse=True).ins
        self.nc.cur_bb.bb.instructions.remove(inst)
        return inst

    def _split_multi_waits(self):
        for fn in self.nc.m.functions:
            for bb in fn.blocks:
                snapshot = list(bb.instructions)
                if not any(
                    inst.sync_info and len(inst.sync_info.on_wait) > 1
                    for inst in snapshot
                ):
                    continue
                new = []
                for inst in snapshot:
                    si = inst.sync_info
                    if si is not None and si.on_wait and len(si.on_wait) > 1:
                        waits = list(si.on_wait)
                        si.on_wait = waits[-1:]
                        inst.sync_info = si
                        for wt in waits[:-1]:
                            nop = self._fresh_nop(inst.engine)
                            nop.sync_info = mybir.SyncInfo(on_wait=[wt], on_update=[])
                            new.append(nop)
                    new.append(inst)
                bb.instructions[:] = new


def build_nc():
    nc = bass.Bass("TRN2")
    wf = nc.dram_tensor("wf", [WPC, AA], _f32, kind="ExternalInput")
    tau3_in = nc.dram_tensor("tau3", [SPL, STW], _bf16, kind="ExternalInput")
    sel3_in = nc.dram_tensor("sel3", [SPL, NST * 128], _bf16, kind="ExternalInput")
    w_in = nc.dram_tensor("W", [AA], _f32, kind="ExternalInput")
    out = nc.dram_tensor("out", [G, 128, F], _f32, kind="ExternalOutput")

    with _TC(nc) as tc:
        with (
            tc.tile_pool(name="const", bufs=1) as const,
            tc.tile_pool(name="outs", bufs=4) as outs,
            tc.tile_pool(name="psum", bufs=2, space="PSUM") as psum,
        ):
            # ---- activation-table priming (Exp/Ln share one func set);
            # runs while the DMAs below are in flight ----
            zeros = const.tile([128, 1], _f32)
            nc.vector.memset(zeros, 0.0)
            prime = const.tile([128, 1], _f32)
            nc.scalar.activation(prime, zeros, _AF.Exp)

            # ---- leading loads: worker features (bias folded in as
            # feature column A) on SP, augmented weights broadcast on Pool --
            wf_sb = const.tile([128, G, AA], _f32)
            nc.sync.dma_start(
                out=wf_sb, in_=wf[:].rearrange("(g p) a -> p g a", p=128)
            )
            w_ap = w_in[:]
            w_sb = const.tile([128, AA], _f32)
            nc.gpsimd.dma_start(
                out=w_sb,
                in_=bass.AP(tensor=w_ap.tensor, offset=w_ap.offset, ap=[[0, 128], [1, AA]]),
            )

            # ---- tau 3-term bf16 split [80, 2048] + stripe selectors,
            # column-sliced over the DMA queues so the first stripe's
            # operands land as early as possible ----
            tau3 = const.tile([SPL, STW], _bf16)
            sel3 = const.tile([SPL, NST * 128], _bf16)
            LOADQ = ("sync", "gpsimd", "scalar", "sync")
            for k in range(4):
                eng = getattr(nc, LOADQ[k])
                eng.dma_start(
                    out=sel3[:, k * 4 * 128 : (k + 1) * 4 * 128],
                    in_=sel3_in[:, k * 4 * 128 : (k + 1) * 4 * 128],
                )
                eng.dma_start(
                    out=tau3[:, k * MM : (k + 1) * MM],
                    in_=tau3_in[:, k * MM : (k + 1) * MM],
                )

            # ---- per-worker scalars: a = ln p1 - ln p2, c = ln p2 ----
            x = const.tile([128, G], _f32)
            for g in range(G):
                prod = const.tile([128, AA], _f32, tag=f"prod{g}")
                nc.vector.tensor_mul(prod, wf_sb[:, g, :], w_sb)
                nc.vector.reduce_sum(x[:, g : g + 1], prod, axis=mybir.AxisListType.X)

            # e = exp(-(x + b));  p1 = 1 / (1 + e);  p2 = (1 - p1) / 15
            e = const.tile([128, G], _f32)
            nc.scalar.activation(e, x, _AF.Exp, bias=0.0, scale=-1.0)
            nc.vector.tensor_scalar_add(e, e, 1.0)
            pack = const.tile([128, 2 * G], _f32)
            nc.vector.reciprocal(pack[:, 0:G], e)
            nc.vector.tensor_scalar(
                pack[:, G : 2 * G],
                pack[:, 0:G],
                scalar1=-1.0 / (L - 1),
                scalar2=1.0 / (L - 1),
                op0=mybir.AluOpType.mult,
                op1=mybir.AluOpType.add,
            )
            lp = const.tile([128, 2 * G], _f32)
            nc.scalar.activation(lp, pack, _AF.Ln)
            lp2 = lp[:, G : 2 * G]
            a_sb = const.tile([128, G], _f32)
            nc.vector.tensor_sub(a_sb, lp[:, 0:G], lp2)

            # ---- main loop: selector-matmul bcast -> Exp -> stream out ----
            wr = 1
            for s in range(NST):  # one 2048-col stripe per iteration
                c0 = s * STW
                pt = psum.tile([128, PSW], _f32, tag="pt", name=f"pt{s}")
                for n in reversed(range(PSW // MM)):
                    nc.tensor.matmul(
                        pt[:, n * MM : (n + 1) * MM],
                        sel3[:, s * 128 : (s + 1) * 128],
                        tau3[:, n * MM : (n + 1) * MM],
                        start=True,
                        stop=True,
                    )
                for gi, g in enumerate((0, 1) if s % 2 == 0 else (1, 0)):
                    ot = outs.tile([128, PSW], _f32, tag=f"ot{g}", name=f"ot{g}_{s}")
                    nc.scalar.activation(
                        ot,
                        pt,
                        _AF.Exp,
                        bias=lp2[:, g : g + 1],
                        scale=a_sb[:, g : g + 1],
                    )
                    if s == NST - 1:
                        # split the final writes so the drain only waits on
                        # a quarter-stripe's DMA latency
                        finq = (
                            ("sync", "gpsimd", "sync", "gpsimd")
                            if gi == 0
                            else ("scalar", "sync", "gpsimd", "scalar")
                        )
                        for q in range(4):
                            getattr(nc, finq[q]).dma_start(
                                out=out[g, :, c0 + q * MM : c0 + (q + 1) * MM],
                                in_=ot[:, q * MM : (q + 1) * MM],
                            )
                        wr += 1
                    else:
                        getattr(nc, WRITE_ENGINES[wr % len(WRITE_ENGINES)]).dma_start(
                            out=out[g, :, c0 : c0 + PSW], in_=ot
                        )
                        wr += 1
    return nc


def _tau3_split(tau):
    """Exact 3-term bf16 split of tau [F] -> [80, 2048] (hi@0, mid@32, lo@64)."""
    import ml_dtypes

    bf = ml_dtypes.bfloat16
    hi = tau.astype(bf)
    r1 = tau - hi.astype(np.float32)
    mid = r1.astype(bf)
    lo = (r1 - mid.astype(np.float32)).astype(bf)
    out = np.zeros((SPL, STW), dtype=bf)
    out[0:NST] = hi.reshape(NST, STW)
    out[32 : 32 + NST] = mid.reshape(NST, STW)
    out[64 : 64 + NST] = lo.reshape(NST, STW)
    return out


def _selector3():
    """sel3[k, s*128 + p] = 1 for k in {s, 32+s, 64+s} (sums the 3-term split)."""
    import ml_dtypes

    sel = np.zeros((SPL, NST * 128), dtype=ml_dtypes.bfloat16)
    for s in range(NST):
        for base in (0, 32, 64):
            sel[base + s, s * 128 : (s + 1) * 128] = 1.0
    return sel


_NC = None


def kernel(inputs, W, b, worker_num=WN, task_num=TN, edge_type=L, ability_num=A, **_kw):
    global _NC
    inputs = np.ascontiguousarray(np.asarray(inputs, dtype=np.float32))
    W = np.asarray(W, dtype=np.float32).reshape(A)
    b = np.asarray(b, dtype=np.float32).reshape(1)
    assert inputs.shape == (WN + TN, A)

    wf = np.concatenate(
        [inputs[:WN, :A], np.ones((WN, 1), dtype=np.float32)], axis=1
    )
    W_aug = np.concatenate([W, b]).astype(np.float32)
    tau = np.ascontiguousarray(inputs[WN:, :L].reshape(F))
    tau3 = _tau3_split(tau)
    sel3 = _selector3()

    if _NC is None:
        _NC = build_nc()

    in_maps = [
        {
            "wf": np.ascontiguousarray(wf[k * WPC : (k + 1) * WPC]),
            "tau3": tau3,
            "sel3": sel3,
            "W": W_aug,
        }
        for k in range(NCORES)
    ]
    res = run_bass_kernel_spmd(_NC, in_maps, core_ids=list(range(NCORES)))
    parts = [r["out"].reshape(WPC, TN, L) for r in res.results]
    return np.concatenate(parts, axis=0)



# revision 4
# speedup vs baseline: 1.6268x; 1.6268x over previous
"""Trainium2 Bass kernel for nn_Decoder (worker/task label-probability decoder).

Math:
    worker_feature = inputs[:2048, :64]          # [Wn, A]
    tau            = inputs[2048:, :16]          # [T, L]
    p1 = sigmoid(worker_feature @ W + b)         # [Wn, 1]
    p2 = (1 - p1) / (L - 1)
    P[i, j, l] = p1[i]^tau[j,l] * p2[i]^(1 - tau[j,l])
               = exp(a[i] * tau[j,l] + c[i]),  a = ln p1 - ln p2, c = ln p2

Sharding: pure data parallel over the worker axis (dim 0), 256 workers per
core across 8 cores; tau replicated. No communication.

Per-core strategy (256 workers = 2 partition groups of 128; F = 32768 task
columns in 32 stripes of 1024):

  * Table path (most stripes): tau is quantized host-side to 4 bits
    (k = floor(16 tau)) with centered remainder zc = 16 tau - k - 0.5 in
    [-0.5, 0.5].  exp(a tau + c) = T[k] * exp(a zc / 16) with
    T[k] = exp(c + a (k+0.5)/16), and the second factor is a 4-term Taylor
    series.  That makes P an exact matmul: lhsT[64, 128] holds
    {T, T a/16, T (a/16)^2/2, T (a/16)^3/6} per worker and the host-built
    rhs[64, cols] holds {onehot(k), onehot*zc, onehot*zc^2, onehot*zc^3}.
    One [64x128] x [64x512] bf16 matmul per 512 output columns; worst-case
    rel err ~0.6% against the 2e-2 budget.
  * Exp path (first SA stripes): baseline-style 2-term bf16 tau split is
    broadcast to all 128 partitions by a selector matmul, and ACT computes
    Exp(a*tau + c) with per-partition scale/bias while evacuating PSUM.
    This offloads the tensor engine (bcast costs half a table matmul).

  PSUM tiles ([128, 1024], 4 rotating) are drained to SBUF staging by
  ACT / DVE / Pool in a weighted round-robin; completed [128, 4096] staging
  slots stream to HBM as strided descriptor writes on the SP queue.  The
  output DRAM layout is chunk-strided (129-float stride per 128-float chunk);
  the host de-stripes and reassembles the [2048, 2048, 16] result.
"""

import numpy as np

try:
    import concourse.bass as bass  # noqa: F401
except ImportError:  # fall back to the container's repo checkout
    import sys

    for _p in ("/root/.axon_site/_ro/trn_rl_repo", "/opt/trn_rl_repo"):
        if _p not in sys.path:
            sys.path.append(_p)

import concourse.bass as bass
import concourse.tile as tile
from concourse import mybir
from concourse.bass_utils import run_bass_kernel_spmd

WN = 2048  # workers total
TN = 2048  # tasks
L = 16  # edge types / labels
A = 64  # ability features
NCORES = 8
WPC = WN // NCORES  # workers per core (256)
G = WPC // 128  # partition groups per core (2)
F = TN * L  # flattened task axis (32768)

NSTR = 32  # stripes of the task axis
STW = F // NSTR  # stripe width (1024)
SA = 6  # stripes handled by the ACT-exp path (rest: table matmul)
NT = NSTR - SA  # table stripes
MM = 512  # matmul columns per instruction

KQ = 16  # tau quantization levels (4 bits)
NTERM = 4  # Taylor terms on the centered remainder
KR = KQ * NTERM  # table contraction rows (64)

SLOTW = 4 * STW  # staging slot width (4096 cols)
NSLOT = NSTR // 4  # staging slots per group (8)
CHUNK = 128  # elements per output descriptor chunk
CSTRIDE = 129  # output chunk stride (1 pad element per chunk)
SEGCH = 128 * SLOTW // CHUNK  # chunks per staging slot (4096)
SEGSZ = SEGCH * CSTRIDE  # padded output elements per slot segment
NSEG = G * NSLOT  # segments total (16)
OUTSZ = NSEG * SEGSZ + CHUNK

_AF = mybir.ActivationFunctionType
_f32 = mybir.dt.float32
_bf16 = mybir.dt.bfloat16


class _TC(tile.TileContext):
    """TileContext legalized for a walrus that allows one sync-wait per inst.

    After Tile's normal scheduling + the exit drain/barrier, rewrite every
    multi-wait instruction into a chain of same-engine NOPs (one wait each)
    followed by the instruction with the final wait.
    """

    def _drain_and_barrier(self, tick_clock, wait_clock):
        super()._drain_and_barrier(tick_clock, wait_clock)
        self._split_multi_waits()

    def _fresh_nop(self, engine):
        inst = self.nc.engines[engine].nop(nofuse=True).ins
        self.nc.cur_bb.bb.instructions.remove(inst)
        return inst

    def _split_multi_waits(self):
        for fn in self.nc.m.functions:
            for bb in fn.blocks:
                snapshot = list(bb.instructions)
                if not any(
                    inst.sync_info and len(inst.sync_info.on_wait) > 1
                    for inst in snapshot
                ):
                    continue
                new = []
                for inst in snapshot:
                    si = inst.sync_info
                    if si is not None and si.on_wait and len(si.on_wait) > 1:
                        waits = list(si.on_wait)
                        si.on_wait = waits[-1:]
                        inst.sync_info = si
                        for wt in waits[:-1]:
                            nop = self._fresh_nop(inst.engine)
                            nop.sync_info = mybir.SyncInfo(on_wait=[wt], on_update=[])
                            new.append(nop)
                    new.append(inst)
                bb.instructions[:] = new


def _seg_ap(out_handle, seg):
    """Strided chunk AP for one staging-slot store."""
    o = out_handle[:]
    return bass.AP(
        tensor=o.tensor,
        offset=o.offset + seg * SEGSZ,
        ap=[[CSTRIDE, SEGCH], [1, CHUNK]],
    )


def build_nc():
    nc = bass.Bass("TRN2")
    lhsT_in = nc.dram_tensor("lhsT", [2 * KR, G * 128], _bf16, kind="ExternalInput")
    rhs_in = nc.dram_tensor("rhs", [128, NT * MM], _bf16, kind="ExternalInput")
    tau3_in = nc.dram_tensor("tau3", [2 * NSTR, STW], _bf16, kind="ExternalInput")
    sel_in = nc.dram_tensor("sel", [2 * NSTR, SA * 128], _bf16, kind="ExternalInput")
    ac_in = nc.dram_tensor("ac", [128, 2 * G], _f32, kind="ExternalInput")
    out = nc.dram_tensor("out", [OUTSZ], _f32, kind="ExternalOutput")

    # weighted evacuation pattern for table units: DVE x2, Pool x3, ACT x1
    EVAC = ("vector", "gpsimd", "gpsimd", "vector", "gpsimd", "scalar")

    with _TC(nc) as tc:
        with (
            tc.tile_pool(name="const", bufs=1) as const,
            tc.tile_pool(name="stg", bufs=2 * G) as stg,
            tc.tile_pool(name="psum", bufs=4, space="PSUM") as psum,
        ):
            # ---- activation-table priming (runs while loads are in flight) --
            zeros = const.tile([128, 1], _f32)
            nc.vector.memset(zeros, 0.0)
            prime = const.tile([128, 1], _f32)
            nc.scalar.activation(prime, zeros, _AF.Exp)

            # ---- leading loads on the SP queue ----
            ac = const.tile([128, 2 * G], _f32)
            nc.sync.dma_start(out=ac, in_=ac_in[:])
            tau3 = const.tile([2 * NSTR, STW], _bf16)
            nc.sync.dma_start(out=tau3, in_=tau3_in[:])
            sel = const.tile([2 * NSTR, SA * 128], _bf16)
            nc.sync.dma_start(out=sel, in_=sel_in[:])
            lhsT = const.tile([2 * KR, G * 128], _bf16)
            nc.sync.dma_start(out=lhsT, in_=lhsT_in[:])
            rhs = const.tile([128, NT * MM], _bf16)
            NLC = 4  # rhs load chunks
            lcw = NT * MM // NLC
            for k in range(NLC):
                nc.sync.dma_start(
                    out=rhs[:, k * lcw : (k + 1) * lcw],
                    in_=rhs_in[:, k * lcw : (k + 1) * lcw],
                )

            slots = [[None] * G for _ in range(NSLOT)]
            evac_i = 0
            seg = 0
            for s in range(NSTR):
                t = s // 4
                off = (s % 4) * STW
                if s % 4 == 0:
                    for g in range(G):
                        slots[t][g] = stg.tile(
                            [128, SLOTW], _f32, tag=f"stg{g}", name=f"stg_{g}_{t}"
                        )
                if s < SA:
                    # ---- exp stripe: selector-matmul bcast -> ACT Exp ----
                    pt = psum.tile([128, STW], _f32, tag="pt", name=f"ptb{s}")
                    for h in range(2):
                        nc.tensor.matmul(
                            pt[:, h * MM : (h + 1) * MM],
                            sel[:, s * 128 : (s + 1) * 128],
                            tau3[:, h * MM : (h + 1) * MM],
                            start=True,
                            stop=True,
                        )
                    for g in range(G):
                        nc.scalar.activation(
                            slots[t][g][:, off : off + STW],
                            pt,
                            _AF.Exp,
                            bias=ac[:, G + g : G + g + 1],
                            scale=ac[:, g : g + 1],
                        )
                else:
                    # ---- table stripe: two 512-col matmuls per group ----
                    ti = s - SA
                    for g in range(G):
                        pt = psum.tile([128, STW], _f32, tag="pt", name=f"pt{s}_{g}")
                        for h in range(2):
                            nc.tensor.matmul(
                                pt[:, h * MM : (h + 1) * MM],
                                lhsT[h * KR : (h + 1) * KR, g * 128 : (g + 1) * 128],
                                rhs[h * KR : (h + 1) * KR, ti * MM : (ti + 1) * MM],
                                start=True,
                                stop=True,
                            )
                        eng = getattr(nc, EVAC[evac_i % len(EVAC)])
                        evac_i += 1
                        dst = slots[t][g][:, off : off + STW]
                        if eng is nc.scalar:
                            nc.scalar.copy(dst, pt)
                        else:
                            eng.tensor_copy(dst, pt)
                if s % 4 == 3:
                    for g in range(G):
                        nc.sync.dma_start(out=_seg_ap(out, seg), in_=slots[t][g][:])
                        seg += 1
    return nc


def _host_tables(inputs_np, W, b):
    """Per-worker scalars a, c and the bf16 table/selector/rhs operands."""
    import ml_dtypes

    bf = ml_dtypes.bfloat16
    wf = inputs_np[:WN, :A].astype(np.float64)
    x = wf @ W.astype(np.float64).reshape(A) + float(b.reshape(1)[0])
    p1 = 1.0 / (1.0 + np.exp(-x))
    p2 = (1.0 - p1) / (L - 1)
    a = np.log(p1) - np.log(p2)  # [WN]
    c = np.log(p2)

    tau = inputs_np[WN:, :L].reshape(F).astype(np.float64)
    q = np.minimum((tau * KQ).astype(np.int64), KQ - 1)
    zc = tau * KQ - q - 0.5  # [-0.5, 0.5]

    # rhs rows: onehot * zc^m  (m = 0..3), [KR, F]
    rhs = np.zeros((KR, F), dtype=bf)
    cols = np.arange(F)
    zpow = np.ones(F, dtype=np.float64)
    for m in range(NTERM):
        rhs[m * KQ + q, cols] = zpow.astype(np.float32)
        zpow = zpow * zc

    # lhsT per worker: rows m*KQ + k = T(k) * (a/16)^m / m!, [KR, WN]
    k = (np.arange(KQ) + 0.5) / KQ  # [KQ]
    T = np.exp(c[None, :] + a[None, :] * k[:, None])  # [KQ, WN]
    fact = np.array([1.0, 1.0, 2.0, 6.0])
    lhsT = np.empty((KR, WN), dtype=bf)
    for m in range(NTERM):
        lhsT[m * KQ : (m + 1) * KQ] = (
            T * ((a[None, :] / KQ) ** m) / fact[m]
        ).astype(bf)

    # 2-term bf16 split of tau, stripes on rows: [2*NSTR, STW]
    tau32 = tau.astype(np.float32)
    hi = tau32.astype(bf)
    mid = (tau32 - hi.astype(np.float32)).astype(bf)
    tau3 = np.zeros((2 * NSTR, STW), dtype=bf)
    tau3[0:NSTR] = hi.reshape(NSTR, STW)
    tau3[NSTR:] = mid.reshape(NSTR, STW)

    # selector for the exp stripes: rows {s, NSTR+s} = 1
    sel = np.zeros((2 * NSTR, SA * 128), dtype=bf)
    for s in range(SA):
        sel[s, s * 128 : (s + 1) * 128] = 1.0
        sel[NSTR + s, s * 128 : (s + 1) * 128] = 1.0

    return a.astype(np.float32), c.astype(np.float32), rhs, lhsT, tau3, sel


def _pack_rhs(rhs):
    """[KR, F] -> [128, NT*512]: table-stripe columns, two vertical halves."""
    packed = np.zeros((128, NT * MM), dtype=rhs.dtype)
    for ti in range(NT):
        s = SA + ti
        c0 = s * STW
        packed[0:KR, ti * MM : (ti + 1) * MM] = rhs[:, c0 : c0 + MM]
        packed[KR:128, ti * MM : (ti + 1) * MM] = rhs[:, c0 + MM : c0 + 2 * MM]
    return packed


_NC = None


def kernel(inputs, W, b, worker_num=WN, task_num=TN, edge_type=L, ability_num=A, **_kw):
    global _NC
    inputs = np.ascontiguousarray(np.asarray(inputs, dtype=np.float32))
    W = np.asarray(W, dtype=np.float32).reshape(A)
    b = np.asarray(b, dtype=np.float32).reshape(1)
    assert inputs.shape == (WN + TN, A)

    a, c, rhs, lhsT, tau3, sel = _host_tables(inputs, W, b)
    rhs_packed = np.ascontiguousarray(_pack_rhs(rhs))

    if _NC is None:
        _NC = build_nc()

    in_maps = []
    for core in range(NCORES):
        w0 = core * WPC
        lhsT_core = np.ascontiguousarray(
            np.concatenate([lhsT[:, w0 : w0 + WPC]] * 2, axis=0)
        )  # [2*KR, 256]
        ac = np.empty((128, 2 * G), dtype=np.float32)
        for g in range(G):
            ac[:, g] = a[w0 + g * 128 : w0 + (g + 1) * 128]
            ac[:, G + g] = c[w0 + g * 128 : w0 + (g + 1) * 128]
        in_maps.append(
            {
                "lhsT": lhsT_core,
                "rhs": rhs_packed,
                "tau3": tau3,
                "sel": sel,
                "ac": ac,
            }
        )

    res = run_bass_kernel_spmd(_NC, in_maps, core_ids=list(range(NCORES)))

    parts = []
    for r in res.results:
        flat = np.asarray(r["out"])
        pc = np.empty((WPC, F), dtype=np.float32)
        for seg in range(NSEG):
            t, g = divmod(seg, G)
            blk = flat[seg * SEGSZ : seg * SEGSZ + SEGCH * CSTRIDE]
            blk = blk.reshape(SEGCH, CSTRIDE)[:, :CHUNK].reshape(128, SLOTW)
            pc[g * 128 : (g + 1) * 128, t * SLOTW : (t + 1) * SLOTW] = blk
        parts.append(pc)
    return np.concatenate(parts, axis=0).reshape(WN, TN, L)


# revision 5
# speedup vs baseline: 1.7587x; 1.0811x over previous
"""Trainium2 Bass kernel for nn_Decoder (worker/task label-probability decoder).

Math:
    worker_feature = inputs[:2048, :64]          # [Wn, A]
    tau            = inputs[2048:, :16]          # [T, L]
    p1 = sigmoid(worker_feature @ W + b)         # [Wn, 1]
    p2 = (1 - p1) / (L - 1)
    P[i, j, l] = p1[i]^tau[j,l] * p2[i]^(1 - tau[j,l])
               = exp(a[i] * tau[j,l] + c[i]),  a = ln p1 - ln p2, c = ln p2

Sharding: pure data parallel over the worker axis (dim 0), 256 workers per
core across 8 cores; tau replicated. No communication.

Per-core strategy (256 workers = 2 partition groups of 128; F = 32768 task
columns in 32 stripes of 1024):

  * Table path (most stripes): tau is quantized host-side to 4 bits
    (k = floor(16 tau)) with centered remainder zc = 16 tau - k - 0.5 in
    [-0.5, 0.5].  exp(a tau + c) = T[k] * exp(a zc / 16) with
    T[k] = exp(c + a (k+0.5)/16), and the second factor is a 4-term Taylor
    series.  That makes P an exact matmul: lhsT[64, 128] holds
    {T, T a/16, T (a/16)^2/2, T (a/16)^3/6} per worker and the host-built
    rhs[64, cols] holds {onehot(k), onehot*zc, onehot*zc^2, onehot*zc^3}.
    One [64x128] x [64x512] bf16 matmul per 512 output columns; worst-case
    rel err ~0.6% against the 2e-2 budget.
  * Exp path (first SA stripes): baseline-style 2-term bf16 tau split is
    broadcast to all 128 partitions by a selector matmul, and ACT computes
    Exp(a*tau + c) with per-partition scale/bias while evacuating PSUM.
    This offloads the tensor engine (bcast costs half a table matmul).

  PSUM tiles ([128, 1024], 4 rotating) are drained to SBUF staging by
  ACT / DVE / Pool in a weighted round-robin; completed [128, 4096] staging
  slots stream to HBM as strided descriptor writes on the SP queue.  The
  output DRAM layout is chunk-strided (129-float stride per 128-float chunk);
  the host de-stripes and reassembles the [2048, 2048, 16] result.
"""

import numpy as np

try:
    import concourse.bass as bass  # noqa: F401
except ImportError:  # fall back to the container's repo checkout
    import sys

    for _p in ("/root/.axon_site/_ro/trn_rl_repo", "/opt/trn_rl_repo"):
        if _p not in sys.path:
            sys.path.append(_p)

import concourse.bass as bass
import concourse.tile as tile
from concourse import mybir
from concourse.bass_utils import run_bass_kernel_spmd

WN = 2048  # workers total
TN = 2048  # tasks
L = 16  # edge types / labels
A = 64  # ability features
NCORES = 8
WPC = WN // NCORES  # workers per core (256)
G = WPC // 128  # partition groups per core (2)
F = TN * L  # flattened task axis (32768)

NSTR = 32  # stripes of the task axis
STW = F // NSTR  # stripe width (1024)
SA = 6  # stripes handled by the ACT-exp path (rest: table matmul)
NT = NSTR - SA  # table stripes
EXP_STRIPES = tuple(2 + 5 * i for i in range(SA))  # spread: 2,7,12,17,22,27
TAB_STRIPES = tuple(s for s in range(NSTR) if s not in EXP_STRIPES)
TAB_IDX = {s: i for i, s in enumerate(TAB_STRIPES)}
EXP_IDX = {s: i for i, s in enumerate(EXP_STRIPES)}
MM = 512  # matmul columns per instruction

KQ = 16  # tau quantization levels (4 bits)
NTERM = 4  # Taylor terms on the centered remainder
KR = KQ * NTERM  # table contraction rows (64)

SLOTW = 4 * STW  # staging slot width (4096 cols)
NSLOT = NSTR // 4  # staging slots per group (8)
CHUNK = 128  # elements per output descriptor chunk
CSTRIDE = 129  # output chunk stride (1 pad element per chunk)
SEGCH = 128 * SLOTW // CHUNK  # chunks per staging slot (4096)
SEGSZ = SEGCH * CSTRIDE  # padded output elements per slot segment
NSEG = G * NSLOT  # segments total (16)
OUTSZ = NSEG * SEGSZ + CHUNK

_AF = mybir.ActivationFunctionType
_f32 = mybir.dt.float32
_bf16 = mybir.dt.bfloat16


class _TC(tile.TileContext):
    """TileContext legalized for a walrus that allows one sync-wait per inst.

    After Tile's normal scheduling + the exit drain/barrier, rewrite every
    multi-wait instruction into a chain of same-engine NOPs (one wait each)
    followed by the instruction with the final wait.
    """

    def _drain_and_barrier(self, tick_clock, wait_clock):
        super()._drain_and_barrier(tick_clock, wait_clock)
        self._split_multi_waits()

    def _fresh_nop(self, engine):
        inst = self.nc.engines[engine].nop(nofuse=True).ins
        self.nc.cur_bb.bb.instructions.remove(inst)
        return inst

    def _split_multi_waits(self):
        for fn in self.nc.m.functions:
            for bb in fn.blocks:
                snapshot = list(bb.instructions)
                if not any(
                    inst.sync_info and len(inst.sync_info.on_wait) > 1
                    for inst in snapshot
                ):
                    continue
                new = []
                for inst in snapshot:
                    si = inst.sync_info
                    if si is not None and si.on_wait and len(si.on_wait) > 1:
                        waits = list(si.on_wait)
                        si.on_wait = waits[-1:]
                        inst.sync_info = si
                        for wt in waits[:-1]:
                            nop = self._fresh_nop(inst.engine)
                            nop.sync_info = mybir.SyncInfo(on_wait=[wt], on_update=[])
                            new.append(nop)
                    new.append(inst)
                bb.instructions[:] = new


def _seg_ap(out_handle, seg):
    """Strided chunk AP for one staging-slot store."""
    o = out_handle[:]
    return bass.AP(
        tensor=o.tensor,
        offset=o.offset + seg * SEGSZ,
        ap=[[CSTRIDE, SEGCH], [1, CHUNK]],
    )


def build_nc():
    nc = bass.Bass("TRN2")
    lhsT_in = nc.dram_tensor("lhsT", [2 * KR, G * 128], _bf16, kind="ExternalInput")
    rhs_in = nc.dram_tensor("rhs", [128, NT * MM], _bf16, kind="ExternalInput")
    tau3_in = nc.dram_tensor("tau3", [2 * NSTR, STW], _bf16, kind="ExternalInput")
    sel_in = nc.dram_tensor("sel", [2 * NSTR, SA * 128], _bf16, kind="ExternalInput")
    ac_in = nc.dram_tensor("ac", [128, 2 * G], _f32, kind="ExternalInput")
    out = nc.dram_tensor("out", [OUTSZ], _f32, kind="ExternalOutput")

    # weighted evacuation pattern for table units: DVE x2, Pool x3, ACT x1
    EVAC = ("vector", "gpsimd", "gpsimd", "vector", "gpsimd", "scalar")

    with _TC(nc) as tc:
        with (
            tc.tile_pool(name="const", bufs=1) as const,
            tc.tile_pool(name="stg", bufs=2 * G) as stg,
            tc.tile_pool(name="psum", bufs=4, space="PSUM") as psum,
        ):
            # ---- activation-table priming (runs while loads are in flight) --
            zeros = const.tile([128, 1], _f32)
            nc.vector.memset(zeros, 0.0)
            prime = const.tile([128, 1], _f32)
            nc.scalar.activation(prime, zeros, _AF.Exp)

            # ---- leading loads, spread across the three DMA queues ----
            ac = const.tile([128, 2 * G], _f32)
            nc.scalar.dma_start(out=ac, in_=ac_in[:])
            lhsT = const.tile([2 * KR, G * 128], _bf16)
            nc.scalar.dma_start(out=lhsT, in_=lhsT_in[:])
            tau3 = const.tile([2 * NSTR, STW], _bf16)
            nc.gpsimd.dma_start(out=tau3, in_=tau3_in[:])
            sel = const.tile([2 * NSTR, SA * 128], _bf16)
            nc.gpsimd.dma_start(out=sel, in_=sel_in[:])
            rhs = const.tile([128, NT * MM], _bf16)
            ledges = [0, 2 * MM, 7 * MM, 16 * MM, NT * MM]  # growing chunks
            for k in range(len(ledges) - 1):
                nc.sync.dma_start(
                    out=rhs[:, ledges[k] : ledges[k + 1]],
                    in_=rhs_in[:, ledges[k] : ledges[k + 1]],
                )

            slots = [[None] * G for _ in range(NSLOT)]
            evac_i = 0
            seg = 0
            for s in range(NSTR):
                t = s // 4
                off = (s % 4) * STW
                if s % 4 == 0:
                    for g in range(G):
                        slots[t][g] = stg.tile(
                            [128, SLOTW], _f32, tag=f"stg{g}", name=f"stg_{g}_{t}"
                        )
                if s in EXP_IDX:
                    # ---- exp stripe: selector-matmul bcast -> ACT Exp ----
                    sa = EXP_IDX[s]
                    pt = psum.tile([128, STW], _f32, tag="pt", name=f"ptb{s}")
                    for h in range(2):
                        nc.tensor.matmul(
                            pt[:, h * MM : (h + 1) * MM],
                            sel[:, sa * 128 : (sa + 1) * 128],
                            tau3[:, h * MM : (h + 1) * MM],
                            start=True,
                            stop=True,
                        )
                    for g in range(G):
                        nc.scalar.activation(
                            slots[t][g][:, off : off + STW],
                            pt,
                            _AF.Exp,
                            bias=ac[:, G + g : G + g + 1],
                            scale=ac[:, g : g + 1],
                        )
                else:
                    # ---- table stripe: two 512-col matmuls per group ----
                    ti = TAB_IDX[s]
                    for g in range(G):
                        pt = psum.tile([128, STW], _f32, tag="pt", name=f"pt{s}_{g}")
                        for h in range(2):
                            nc.tensor.matmul(
                                pt[:, h * MM : (h + 1) * MM],
                                lhsT[h * KR : (h + 1) * KR, g * 128 : (g + 1) * 128],
                                rhs[h * KR : (h + 1) * KR, ti * MM : (ti + 1) * MM],
                                start=True,
                                stop=True,
                            )
                        eng = getattr(nc, EVAC[evac_i % len(EVAC)])
                        evac_i += 1
                        dst = slots[t][g][:, off : off + STW]
                        if eng is nc.scalar:
                            nc.scalar.copy(dst, pt)
                        else:
                            eng.tensor_copy(dst, pt)
                if s % 4 == 3:
                    for g in range(G):
                        nc.sync.dma_start(out=_seg_ap(out, seg), in_=slots[t][g][:])
                        seg += 1
    return nc


def _host_tables(inputs_np, W, b):
    """Per-worker scalars a, c and the bf16 table/selector/rhs operands."""
    import ml_dtypes

    bf = ml_dtypes.bfloat16
    wf = inputs_np[:WN, :A].astype(np.float64)
    x = wf @ W.astype(np.float64).reshape(A) + float(b.reshape(1)[0])
    p1 = 1.0 / (1.0 + np.exp(-x))
    p2 = (1.0 - p1) / (L - 1)
    a = np.log(p1) - np.log(p2)  # [WN]
    c = np.log(p2)

    tau = inputs_np[WN:, :L].reshape(F).astype(np.float64)
    q = np.minimum((tau * KQ).astype(np.int64), KQ - 1)
    zc = tau * KQ - q - 0.5  # [-0.5, 0.5]

    # rhs rows: onehot * zc^m  (m = 0..3), [KR, F]
    rhs = np.zeros((KR, F), dtype=bf)
    cols = np.arange(F)
    zpow = np.ones(F, dtype=np.float64)
    for m in range(NTERM):
        rhs[m * KQ + q, cols] = zpow.astype(np.float32)
        zpow = zpow * zc

    # lhsT per worker: rows m*KQ + k = T(k) * (a/16)^m / m!, [KR, WN]
    k = (np.arange(KQ) + 0.5) / KQ  # [KQ]
    T = np.exp(c[None, :] + a[None, :] * k[:, None])  # [KQ, WN]
    fact = np.array([1.0, 1.0, 2.0, 6.0])
    lhsT = np.empty((KR, WN), dtype=bf)
    for m in range(NTERM):
        lhsT[m * KQ : (m + 1) * KQ] = (
            T * ((a[None, :] / KQ) ** m) / fact[m]
        ).astype(bf)

    # 2-term bf16 split of tau, stripes on rows: [2*NSTR, STW]
    tau32 = tau.astype(np.float32)
    hi = tau32.astype(bf)
    mid = (tau32 - hi.astype(np.float32)).astype(bf)
    tau3 = np.zeros((2 * NSTR, STW), dtype=bf)
    tau3[0:NSTR] = hi.reshape(NSTR, STW)
    tau3[NSTR:] = mid.reshape(NSTR, STW)

    # selector for the exp stripes: rows {s, NSTR+s} = 1
    sel = np.zeros((2 * NSTR, SA * 128), dtype=bf)
    for sa, s in enumerate(EXP_STRIPES):
        sel[s, sa * 128 : (sa + 1) * 128] = 1.0
        sel[NSTR + s, sa * 128 : (sa + 1) * 128] = 1.0

    return a.astype(np.float32), c.astype(np.float32), rhs, lhsT, tau3, sel


def _pack_rhs(rhs):
    """[KR, F] -> [128, NT*512]: table-stripe columns, two vertical halves."""
    packed = np.zeros((128, NT * MM), dtype=rhs.dtype)
    for ti, s in enumerate(TAB_STRIPES):
        c0 = s * STW
        packed[0:KR, ti * MM : (ti + 1) * MM] = rhs[:, c0 : c0 + MM]
        packed[KR:128, ti * MM : (ti + 1) * MM] = rhs[:, c0 + MM : c0 + 2 * MM]
    return packed


_NC = None


def kernel(inputs, W, b, worker_num=WN, task_num=TN, edge_type=L, ability_num=A, **_kw):
    global _NC
    inputs = np.ascontiguousarray(np.asarray(inputs, dtype=np.float32))
    W = np.asarray(W, dtype=np.float32).reshape(A)
    b = np.asarray(b, dtype=np.float32).reshape(1)
    assert inputs.shape == (WN + TN, A)

    a, c, rhs, lhsT, tau3, sel = _host_tables(inputs, W, b)
    rhs_packed = np.ascontiguousarray(_pack_rhs(rhs))

    if _NC is None:
        _NC = build_nc()

    in_maps = []
    for core in range(NCORES):
        w0 = core * WPC
        lhsT_core = np.ascontiguousarray(
            np.concatenate([lhsT[:, w0 : w0 + WPC]] * 2, axis=0)
        )  # [2*KR, 256]
        ac = np.empty((128, 2 * G), dtype=np.float32)
        for g in range(G):
            ac[:, g] = a[w0 + g * 128 : w0 + (g + 1) * 128]
            ac[:, G + g] = c[w0 + g * 128 : w0 + (g + 1) * 128]
        in_maps.append(
            {
                "lhsT": lhsT_core,
                "rhs": rhs_packed,
                "tau3": tau3,
                "sel": sel,
                "ac": ac,
            }
        )

    res = run_bass_kernel_spmd(_NC, in_maps, core_ids=list(range(NCORES)))

    parts = []
    for r in res.results:
        flat = np.asarray(r["out"])
        pc = np.empty((WPC, F), dtype=np.float32)
        for seg in range(NSEG):
            t, g = divmod(seg, G)
            blk = flat[seg * SEGSZ : seg * SEGSZ + SEGCH * CSTRIDE]
            blk = blk.reshape(SEGCH, CSTRIDE)[:, :CHUNK].reshape(128, SLOTW)
            pc[g * 128 : (g + 1) * 128, t * SLOTW : (t + 1) * SLOTW] = blk
        parts.append(pc)
    return np.concatenate(parts, axis=0).reshape(WN, TN, L)


# revision 7
# speedup vs baseline: 1.9108x; 1.0864x over previous
"""Trainium2 Bass kernel for nn_Decoder (worker/task label-probability decoder).

Math:
    worker_feature = inputs[:2048, :64]          # [Wn, A]
    tau            = inputs[2048:, :16]          # [T, L]
    p1 = sigmoid(worker_feature @ W + b)         # [Wn, 1]
    p2 = (1 - p1) / (L - 1)
    P[i, j, l] = p1[i]^tau[j,l] * p2[i]^(1 - tau[j,l])
               = exp(a[i] * tau[j,l] + c[i]),  a = ln p1 - ln p2, c = ln p2

Sharding: pure data parallel over the worker axis (dim 0), 256 workers per
core across 8 cores; tau replicated. No communication.

Per-core strategy (256 workers = 2 partition groups of 128; F = 32768 task
columns in 32 stripes of 1024):

  * Table path (most stripes): tau is quantized host-side to 4 bits
    (k = floor(16 tau)) with centered remainder zc = 16 tau - k - 0.5 in
    [-0.5, 0.5].  exp(a tau + c) = T[k] * exp(a zc / 16) with
    T[k] = exp(c + a (k+0.5)/16), and the second factor is a 4-term Taylor
    series.  That makes P an exact matmul: lhsT[64, 128] holds
    {T, T a/16, T (a/16)^2/2, T (a/16)^3/6} per worker and the host-built
    rhs[64, cols] holds {onehot(k), onehot*zc, onehot*zc^2, onehot*zc^3}.
    One [64x128] x [64x512] bf16 matmul per 512 output columns; worst-case
    rel err ~0.6% against the 2e-2 budget.
  * Exp path (first SA stripes): baseline-style 2-term bf16 tau split is
    broadcast to all 128 partitions by a selector matmul, and ACT computes
    Exp(a*tau + c) with per-partition scale/bias while evacuating PSUM.
    This offloads the tensor engine (bcast costs half a table matmul).

  PSUM tiles ([128, 1024], 4 rotating) are drained to SBUF staging by
  ACT / DVE / Pool in a weighted round-robin; completed [128, 4096] staging
  slots stream to HBM as strided descriptor writes on the SP queue.  The
  output DRAM layout is chunk-strided (129-float stride per 128-float chunk);
  the host de-stripes and reassembles the [2048, 2048, 16] result.
"""

import numpy as np

try:
    import concourse.bass as bass  # noqa: F401
except ImportError:  # fall back to the container's repo checkout
    import sys

    for _p in ("/root/.axon_site/_ro/trn_rl_repo", "/opt/trn_rl_repo"):
        if _p not in sys.path:
            sys.path.append(_p)

import concourse.bass as bass
import concourse.tile as tile
from concourse import mybir
from concourse.bass_utils import run_bass_kernel_spmd

WN = 2048  # workers total
TN = 2048  # tasks
L = 16  # edge types / labels
A = 64  # ability features
NCORES = 8
WPC = WN // NCORES  # workers per core (256)
G = WPC // 128  # partition groups per core (2)
F = TN * L  # flattened task axis (32768)

NSTR = 32  # stripes of the task axis
STW = F // NSTR  # stripe width (1024)
SA = 7  # stripes handled by the ACT-exp path (rest: table matmul)
NT = NSTR - SA  # table stripes
EXP_STRIPES = tuple(2 + (NSTR - 4) * i // SA for i in range(SA))  # spread
TAB_STRIPES = tuple(s for s in range(NSTR) if s not in EXP_STRIPES)
TAB_IDX = {s: i for i, s in enumerate(TAB_STRIPES)}
EXP_IDX = {s: i for i, s in enumerate(EXP_STRIPES)}
MM = 512  # matmul columns per instruction

KQ = 16  # tau quantization levels (4 bits)
NTERM = 4  # Taylor terms on the centered remainder
KR = KQ * NTERM  # table contraction rows (64)

SLOTW = 4 * STW  # staging slot width (4096 cols)
NSLOT = NSTR // 4  # staging slots per group (8)
CHUNK = 128  # elements per output descriptor chunk
CSTRIDE = 129  # output chunk stride (1 pad element per chunk)
SEGCH = 128 * SLOTW // CHUNK  # chunks per staging slot (4096)
SEGSZ = SEGCH * CSTRIDE  # padded output elements per slot segment
NSEG = G * NSLOT  # segments total (16)
OUTSZ = NSEG * SEGSZ + CHUNK

_AF = mybir.ActivationFunctionType
_f32 = mybir.dt.float32
_bf16 = mybir.dt.bfloat16


class _TC(tile.TileContext):
    """TileContext legalized for a walrus that allows one sync-wait per inst.

    After Tile's normal scheduling + the exit drain/barrier, rewrite every
    multi-wait instruction into a chain of same-engine NOPs (one wait each)
    followed by the instruction with the final wait.
    """

    def _drain_and_barrier(self, tick_clock, wait_clock):
        super()._drain_and_barrier(tick_clock, wait_clock)
        self._split_multi_waits()

    def _fresh_nop(self, engine):
        inst = self.nc.engines[engine].nop(nofuse=True).ins
        self.nc.cur_bb.bb.instructions.remove(inst)
        return inst

    def _split_multi_waits(self):
        for fn in self.nc.m.functions:
            for bb in fn.blocks:
                snapshot = list(bb.instructions)
                if not any(
                    inst.sync_info and len(inst.sync_info.on_wait) > 1
                    for inst in snapshot
                ):
                    continue
                new = []
                for inst in snapshot:
                    si = inst.sync_info
                    if si is not None and si.on_wait and len(si.on_wait) > 1:
                        waits = list(si.on_wait)
                        si.on_wait = waits[-1:]
                        inst.sync_info = si
                        for wt in waits[:-1]:
                            nop = self._fresh_nop(inst.engine)
                            nop.sync_info = mybir.SyncInfo(on_wait=[wt], on_update=[])
                            new.append(nop)
                    new.append(inst)
                bb.instructions[:] = new


def _seg_ap(out_handle, seg):
    """Strided chunk AP for one staging-slot store."""
    o = out_handle[:]
    return bass.AP(
        tensor=o.tensor,
        offset=o.offset + seg * SEGSZ,
        ap=[[CSTRIDE, SEGCH], [1, CHUNK]],
    )


def _sub_ap(out_handle, seg, q):
    """Quarter-slot store (one stripe of a staging slot)."""
    o = out_handle[:]
    return bass.AP(
        tensor=o.tensor,
        offset=o.offset + seg * SEGSZ + q * (SEGCH // 4) * CSTRIDE,
        ap=[[CSTRIDE, SEGCH // 4], [1, CHUNK]],
    )


def build_nc():
    nc = bass.Bass("TRN2")
    lhsT_in = nc.dram_tensor("lhsT", [2 * KR, G * 128], _bf16, kind="ExternalInput")
    rhs_in = nc.dram_tensor("rhs", [128, NT * MM], _bf16, kind="ExternalInput")
    tau3_in = nc.dram_tensor("tau3", [2 * NSTR, STW], _bf16, kind="ExternalInput")
    sel_in = nc.dram_tensor("sel", [2 * NSTR, SA * 128], _bf16, kind="ExternalInput")
    ac_in = nc.dram_tensor("ac", [128, 2 * G], _f32, kind="ExternalInput")
    out = nc.dram_tensor("out", [OUTSZ], _f32, kind="ExternalOutput")

    # weighted evacuation pattern for table units
    EVAC = ("gpsimd", "vector", "gpsimd", "vector", "gpsimd", "scalar",
            "gpsimd", "vector", "gpsimd", "vector")

    with _TC(nc) as tc:
        with (
            tc.tile_pool(name="const", bufs=1) as const,
            tc.tile_pool(name="stg", bufs=2 * G) as stg,
            tc.tile_pool(name="psum", bufs=4, space="PSUM") as psum,
        ):
            # ---- activation-table priming (runs while loads are in flight) --
            zeros = const.tile([128, 1], _f32)
            nc.vector.memset(zeros, 0.0)
            prime = const.tile([128, 1], _f32)
            nc.scalar.activation(prime, zeros, _AF.Exp)

            # ---- leading loads, spread across the three DMA queues ----
            lhsT = const.tile([2 * KR, G * 128], _bf16)
            nc.scalar.dma_start(out=lhsT, in_=lhsT_in[:])
            ac = const.tile([128, 2 * G], _f32)
            nc.scalar.dma_start(out=ac, in_=ac_in[:])
            tau3 = const.tile([2 * NSTR, STW], _bf16)
            nc.scalar.dma_start(out=tau3, in_=tau3_in[:])
            sel = const.tile([2 * NSTR, SA * 128], _bf16)
            nc.scalar.dma_start(out=sel, in_=sel_in[:])
            rhs = const.tile([128, NT * MM], _bf16)
            ledges = [0, MM, 3 * MM, 7 * MM, 12 * MM, 18 * MM, NT * MM]
            for k in range(len(ledges) - 1):
                nc.sync.dma_start(
                    out=rhs[:, ledges[k] : ledges[k + 1]],
                    in_=rhs_in[:, ledges[k] : ledges[k + 1]],
                )

            slots = [[None] * G for _ in range(NSLOT)]
            evac_i = 0
            for s in range(NSTR):
                t = s // 4
                off = (s % 4) * STW
                if s % 4 == 0:
                    for g in range(G):
                        slots[t][g] = stg.tile(
                            [128, SLOTW], _f32, tag=f"stg{g}", name=f"stg_{g}_{t}"
                        )
                if s in EXP_IDX:
                    # ---- exp stripe: selector-matmul bcast -> ACT Exp ----
                    sa = EXP_IDX[s]
                    pt = psum.tile([128, STW], _f32, tag="pt", name=f"ptb{s}")
                    for h in range(2):
                        nc.tensor.matmul(
                            pt[:, h * MM : (h + 1) * MM],
                            sel[:, sa * 128 : (sa + 1) * 128],
                            tau3[:, h * MM : (h + 1) * MM],
                            start=True,
                            stop=True,
                        )
                    for g in range(G):
                        nc.scalar.activation(
                            slots[t][g][:, off : off + STW],
                            pt,
                            _AF.Exp,
                            bias=ac[:, G + g : G + g + 1],
                            scale=ac[:, g : g + 1],
                        )
                else:
                    # ---- table stripe: two 512-col matmuls per group ----
                    ti = TAB_IDX[s]
                    for g in range(G):
                        pt = psum.tile([128, STW], _f32, tag="pt", name=f"pt{s}_{g}")
                        for h in range(2):
                            nc.tensor.matmul(
                                pt[:, h * MM : (h + 1) * MM],
                                lhsT[h * KR : (h + 1) * KR, g * 128 : (g + 1) * 128],
                                rhs[h * KR : (h + 1) * KR, ti * MM : (ti + 1) * MM],
                                start=True,
                                stop=True,
                            )
                        eng = getattr(nc, EVAC[evac_i % len(EVAC)])
                        evac_i += 1
                        dst = slots[t][g][:, off : off + STW]
                        if eng is nc.scalar:
                            nc.scalar.copy(dst, pt)
                        else:
                            eng.tensor_copy(dst, pt)
                if t == NSLOT - 1:
                    # final slot: store each stripe as soon as it drains
                    for g in range(G):
                        nc.sync.dma_start(
                            out=_sub_ap(out, G * t + g, s % 4), in_=slots[t][g][:, off : off + STW]
                        )
                elif s % 4 == 3:
                    for g in range(G):
                        nc.sync.dma_start(out=_seg_ap(out, G * t + g), in_=slots[t][g][:])
    return nc


def _host_tables(inputs_np, W, b):
    """Per-worker scalars a, c and the bf16 table/selector/rhs operands."""
    import ml_dtypes

    bf = ml_dtypes.bfloat16
    wf = inputs_np[:WN, :A].astype(np.float64)
    x = wf @ W.astype(np.float64).reshape(A) + float(b.reshape(1)[0])
    p1 = 1.0 / (1.0 + np.exp(-x))
    p2 = (1.0 - p1) / (L - 1)
    a = np.log(p1) - np.log(p2)  # [WN]
    c = np.log(p2)

    tau = inputs_np[WN:, :L].reshape(F).astype(np.float64)
    q = np.minimum((tau * KQ).astype(np.int64), KQ - 1)
    zc = tau * KQ - q - 0.5  # [-0.5, 0.5]

    # rhs rows: onehot * zc^m  (m = 0..3), [KR, F]
    rhs = np.zeros((KR, F), dtype=bf)
    cols = np.arange(F)
    zpow = np.ones(F, dtype=np.float64)
    for m in range(NTERM):
        rhs[m * KQ + q, cols] = zpow.astype(np.float32)
        zpow = zpow * zc

    # lhsT per worker: rows m*KQ + k = T(k) * (a/16)^m / m!, [KR, WN]
    k = (np.arange(KQ) + 0.5) / KQ  # [KQ]
    T = np.exp(c[None, :] + a[None, :] * k[:, None])  # [KQ, WN]
    fact = np.array([1.0, 1.0, 2.0, 6.0])
    lhsT = np.empty((KR, WN), dtype=bf)
    for m in range(NTERM):
        lhsT[m * KQ : (m + 1) * KQ] = (
            T * ((a[None, :] / KQ) ** m) / fact[m]
        ).astype(bf)

    # 2-term bf16 split of tau, stripes on rows: [2*NSTR, STW]
    tau32 = tau.astype(np.float32)
    hi = tau32.astype(bf)
    mid = (tau32 - hi.astype(np.float32)).astype(bf)
    tau3 = np.zeros((2 * NSTR, STW), dtype=bf)
    tau3[0:NSTR] = hi.reshape(NSTR, STW)
    tau3[NSTR:] = mid.reshape(NSTR, STW)

    # selector for the exp stripes: rows {s, NSTR+s} = 1
    sel = np.zeros((2 * NSTR, SA * 128), dtype=bf)
    for sa, s in enumerate(EXP_STRIPES):
        sel[s, sa * 128 : (sa + 1) * 128] = 1.0
        sel[NSTR + s, sa * 128 : (sa + 1) * 128] = 1.0

    return a.astype(np.float32), c.astype(np.float32), rhs, lhsT, tau3, sel


def _pack_rhs(rhs):
    """[KR, F] -> [128, NT*512]: table-stripe columns, two vertical halves."""
    packed = np.zeros((128, NT * MM), dtype=rhs.dtype)
    for ti, s in enumerate(TAB_STRIPES):
        c0 = s * STW
        packed[0:KR, ti * MM : (ti + 1) * MM] = rhs[:, c0 : c0 + MM]
        packed[KR:128, ti * MM : (ti + 1) * MM] = rhs[:, c0 + MM : c0 + 2 * MM]
    return packed


def _decode_seg(blk_padded, final_slot):
    """[SEGSZ] padded chunk stream -> [128, SLOTW] staging image."""
    chunks = blk_padded.reshape(SEGCH, CSTRIDE)[:, :CHUNK]
    if not final_slot:
        return chunks.reshape(128, SLOTW)
    # final slot was stored stripe-by-stripe: chunk order (q, p, b)
    q = chunks.reshape(4, 128, SLOTW // 4)
    return np.concatenate([q[i] for i in range(4)], axis=1)


_NC = None


def kernel(inputs, W, b, worker_num=WN, task_num=TN, edge_type=L, ability_num=A, **_kw):
    global _NC
    inputs = np.ascontiguousarray(np.asarray(inputs, dtype=np.float32))
    W = np.asarray(W, dtype=np.float32).reshape(A)
    b = np.asarray(b, dtype=np.float32).reshape(1)
    assert inputs.shape == (WN + TN, A)

    a, c, rhs, lhsT, tau3, sel = _host_tables(inputs, W, b)
    rhs_packed = np.ascontiguousarray(_pack_rhs(rhs))

    if _NC is None:
        _NC = build_nc()

    in_maps = []
    for core in range(NCORES):
        w0 = core * WPC
        lhsT_core = np.ascontiguousarray(
            np.concatenate([lhsT[:, w0 : w0 + WPC]] * 2, axis=0)
        )  # [2*KR, 256]
        ac = np.empty((128, 2 * G), dtype=np.float32)
        for g in range(G):
            ac[:, g] = a[w0 + g * 128 : w0 + (g + 1) * 128]
            ac[:, G + g] = c[w0 + g * 128 : w0 + (g + 1) * 128]
        in_maps.append(
            {
                "lhsT": lhsT_core,
                "rhs": rhs_packed,
                "tau3": tau3,
                "sel": sel,
                "ac": ac,
            }
        )

    res = run_bass_kernel_spmd(_NC, in_maps, core_ids=list(range(NCORES)))

    parts = []
    for r in res.results:
        flat = np.asarray(r["out"])
        pc = np.empty((WPC, F), dtype=np.float32)
        for seg in range(NSEG):
            t, g = divmod(seg, G)
            blk = _decode_seg(
                flat[seg * SEGSZ : seg * SEGSZ + SEGCH * CSTRIDE], t == NSLOT - 1
            )
            pc[g * 128 : (g + 1) * 128, t * SLOTW : (t + 1) * SLOTW] = blk
        parts.append(pc)
    return np.concatenate(parts, axis=0).reshape(WN, TN, L)
